# revision 11
# baseline (speedup 1.0000x reference)
"""DeepGCN (GENConv softmax-aggr) Trainium2 kernel, 8-core SPMD.

Sharding: by spatial feature position (H*W = 48 -> 6 per core).
Core k owns positions {h*4 + (k%4) : h in [6*(k//4), 6*(k//4)+6)}.
All nodes are local on every core; per-layer BN stats use a tiny (512B)
AllReduce, and the final mean-over-W pooling uses one fp16 AllReduce
within each h-half group of 4 cores.
"""

import sys
import numpy as np

for p in ("/opt/trn_rl_repo",):
    if p not in sys.path:
        sys.path.insert(0, p)

# ---- problem constants (hardcoded per spec) ----
N_NODES = 5000
N_EDGES = 50000
C = 64
H, W = 12, 4
HW = H * W          # 48
PS = 6              # positions per core
F = C * PS          # 384 features per core-slice  (layout f = p*64 + c)
O = 12
L = 3
NCORES = 8
NCHUNK = 40         # dst-node chunks of 128
NODES_PAD = NCHUNK * 128   # 5120
G = 4               # edge blocks per DMA/compute group (128 edges each)
BN_EPS = 1e-5
GEN_EPS = 1e-7
EXP_BIAS = -4.0     # e' = exp(m + EXP_BIAS); cancels in softmax, keeps fp16 safe
BN_COUNT = float(N_NODES * HW)

_cache = {}
DEBUG = False


def _host_prep(node_feats, edge_feats, src, dst):
    """Sort/pad edges by dst chunk, build per-core input maps."""
    src = np.asarray(src).astype(np.int64)
    dst = np.asarray(dst).astype(np.int64)
    nf = np.asarray(node_feats, dtype=np.float32).reshape(N_NODES, C, HW)
    ef = np.asarray(edge_feats, dtype=np.float32).reshape(N_EDGES, C, HW)

    order = np.argsort(dst, kind="stable")
    chunk_of = dst[order] // 128

    blk_edges = []          # original-edge indices, -1 for pads
    nblk_per_chunk = []
    for c in range(NCHUNK):
        sel = order[chunk_of == c]
        nb = max(1, (len(sel) + 127) // 128)
        pad = nb * 128 - len(sel)
        blk_edges.append(np.concatenate([sel, np.full(pad, -1, np.int64)]))
        nblk_per_chunk.append(nb)
    nblk_tot = sum(nblk_per_chunk)
    extra = (-nblk_tot) % G
    if extra:
        blk_edges[-1] = np.concatenate(
            [blk_edges[-1], np.full(extra * 128, -1, np.int64)]
        )
        nblk_per_chunk[-1] += extra
        nblk_tot += extra

    eidx = np.concatenate(blk_edges)          # [nblk_tot*128]
    valid = eidx >= 0
    e_src = np.where(valid, src[np.maximum(eidx, 0)], 0).astype(np.int32)
    chunk_id = np.concatenate(
        [np.full(nblk_per_chunk[c] * 128, c, np.int64) for c in range(NCHUNK)]
    )
    e_ldst = np.where(
        valid, dst[np.maximum(eidx, 0)] - chunk_id * 128, -1
    ).astype(np.float32)

    NBLK = nblk_tot
    NG = NBLK // G
    gidx_d = np.ascontiguousarray(e_src.reshape(NBLK, 128).T)    # [128, NBLK] int32
    ldst_d = np.ascontiguousarray(e_ldst.reshape(NBLK, 128).T)   # [128, NBLK] f32

    pos_sets = []
    for k in range(NCORES):
        w = k % 4
        h0 = 6 * (k // 4)
        pos_sets.append(np.array([h * 4 + w for h in range(h0, h0 + 6)]))

    ev = eidx[valid]
    in_maps = []
    for k in range(NCORES):
        P_k = pos_sets[k]
        hv0 = np.zeros((NODES_PAD, F), np.float32)
        hv0[:N_NODES] = nf[:, :, P_k].transpose(0, 2, 1).reshape(N_NODES, F)
        he_s = np.zeros((NBLK * 128, F), np.float16)
        he_s[valid] = (
            ef[ev][:, :, P_k].transpose(0, 2, 1).reshape(len(ev), F).astype(np.float16)
        )
        he_pm = (
            he_s.reshape(NG, G, 128, F).transpose(0, 2, 1, 3).reshape(NG, 128, G * F)
        )
        in_maps.append(
            {
                "hv0": hv0,
                "he_pm": np.ascontiguousarray(he_pm),
                "gidx": gidx_d,
                "ldst": ldst_d,
            }
        )
    return in_maps, nblk_per_chunk, pos_sets


def _build_program(nblk_per_chunk):
    import concourse.bacc as bacc
    import concourse.tile as tile
    from concourse import bass, mybir
    from concourse.masks import make_identity

    dt = mybir.dt
    AF = mybir.ActivationFunctionType
    ALU = mybir.AluOpType
    NBLK = sum(nblk_per_chunk)
    NG = NBLK // G

    nc = bacc.Bacc(
        "TRN2",
        target_bir_lowering=False,
        debug=False,
        enable_asserts=False,
        num_devices=NCORES,
    )

    hv0_d = nc.dram_tensor("hv0", [NODES_PAD, F], dt.float32, kind="ExternalInput").ap()
    he_d = nc.dram_tensor(
        "he_pm", [NG, 128, G * F], dt.float16, kind="ExternalInput"
    ).ap()
    gidx_d = nc.dram_tensor("gidx", [128, NBLK], dt.int32, kind="ExternalInput").ap()
    ldst_d = nc.dram_tensor("ldst", [128, NBLK], dt.float32, kind="ExternalInput").ap()
    bng_d = nc.dram_tensor("bn_gamma", [L, C], dt.float32, kind="ExternalInput").ap()
    bnb_d = nc.dram_tensor("bn_beta", [L, C], dt.float32, kind="ExternalInput").ap()
    genw_d = nc.dram_tensor("gen_W", [L * C, C], dt.float32, kind="ExternalInput").ap()
    genb_d = nc.dram_tensor("gen_b", [L, C], dt.float32, kind="ExternalInput").ap()
    outw_d = nc.dram_tensor("out_W", [C, O], dt.float32, kind="ExternalInput").ap()
    outb_d = nc.dram_tensor("out_b", [O, 1], dt.float32, kind="ExternalInput").ap()
    out_d = nc.dram_tensor(
        "out_sh", [NODES_PAD, O * PS], dt.float32, kind="ExternalOutput"
    ).ap()
    if DEBUG:
        dbg_h1 = nc.dram_tensor(
            "dbg_h1", [NODES_PAD, F], dt.float16, kind="ExternalOutput"
        ).ap()
        dbg_hv1 = nc.dram_tensor(
            "dbg_hv1", [128, NCHUNK * F], dt.float32, kind="ExternalOutput"
        ).ap()
        dbg_feats0 = nc.dram_tensor(
            "dbg_feats0", [128, F], dt.float32, kind="ExternalOutput"
        ).ap()
        dbg_gath0 = nc.dram_tensor(
            "dbg_gath0", [128, G * F], dt.float16, kind="ExternalOutput"
        ).ap()

    with tile.TileContext(nc) as tc:
        with (
            tc.tile_pool(name="dram", bufs=1, space="DRAM") as dramp,
            tc.tile_pool(name="resident", bufs=1) as res,
            tc.tile_pool(name="wf32", bufs=2) as wf32,
            tc.tile_pool(name="edge", bufs=2) as edgep,
            tc.tile_pool(name="small", bufs=2) as small,
            tc.tile_pool(name="psA", bufs=4, space="PSUM") as psA,
            tc.tile_pool(name="psB", bufs=3, space="PSUM") as psB,
        ):
            h1t = dramp.tile([NODES_PAD, F], dt.float16, name="h1t")
            bn_in = dramp.tile([1, 128], dt.float32, name="bn_in")
            bn_outs = [
                dramp.tile(
                    [1, 128], dt.float32, addr_space="Shared",
                    name=f"bn_out{l}", tag=f"bn_out{l}",
                )
                for l in range(L)
            ]
            mw_in = dramp.tile([NODES_PAD, F], dt.float16, name="mw_in")
            mw_out = dramp.tile([NODES_PAD, F], dt.float16, name="mw_out")

            hv = res.tile([128, NCHUNK * F], dt.float32, name="hv")
            h1f = res.tile([128, NCHUNK * F], dt.float32, name="h1f")
            gidx_sb = res.tile([128, NBLK], dt.int32, name="gidx_sb")
            ldst_sb = res.tile([128, NBLK], dt.float32, name="ldst_sb")
            ident = res.tile([128, 128], dt.float32, name="ident")
            iota_f = res.tile([128, 128], dt.float32, name="iota_f")
            ones_col = res.tile([128, 1], dt.float32, name="ones_col")
            ones_row = res.tile([1, 128], dt.float32, name="ones_row")
            genw_sb = res.tile([C, L * C], dt.float32, name="genw_sb")
            genb_sb = res.tile([C, L], dt.float32, name="genb_sb")
            outw_sb = res.tile([C, O], dt.float32, name="outw_sb")
            outb_sb = res.tile([O, 1], dt.float32, name="outb_sb")
            bngam = res.tile([1, L * C], dt.float32, name="bngam")
            bnbet = res.tile([1, L * C], dt.float32, name="bnbet")
            a_bc = res.tile([128, F], dt.float32, name="a_bc")
            b_bc = res.tile([128, F], dt.float32, name="b_bc")
            eps_bn = res.tile([1, 1], dt.float32, name="eps_bn")
            pad_mask = res.tile([128, 1], dt.float32, name="pad_mask")
            ebias = res.tile([128, 1], dt.float32, name="ebias")
            s_floor = res.tile([128, 1], dt.float32, name="s_floor")

            make_identity(nc, ident[:])
            iota_i = small.tile([128, 128], dt.int32, name="iota_i", tag="ioi")
            nc.gpsimd.iota(iota_i[:], pattern=[[1, 128]], base=0, channel_multiplier=0)
            nc.vector.tensor_copy(iota_f[:], iota_i[:])
            nc.gpsimd.memset(ones_col[:], 1.0)
            nc.gpsimd.memset(ones_row[:], 1.0)
            nc.gpsimd.memset(eps_bn[:], BN_EPS)
            nc.gpsimd.memset(ebias[:], EXP_BIAS)
            nc.gpsimd.memset(s_floor[:], 1e-30)
            iota_c = small.tile([128, 1], dt.int32, name="iota_c", tag="ioc")
            nc.gpsimd.iota(iota_c[:], pattern=[[1, 1]], base=0, channel_multiplier=1)
            nc.vector.tensor_scalar(
                out=pad_mask[:], in0=iota_c[:], scalar1=8, scalar2=None,
                op0=ALU.is_lt,
            )

            nc.sync.dma_start(gidx_sb[:], gidx_d)
            nc.sync.dma_start(ldst_sb[:], ldst_d)
            for l in range(L):
                nc.sync.dma_start(
                    genw_sb[:, l * C : (l + 1) * C], genw_d[l * C : (l + 1) * C, :]
                )
            nc.sync.dma_start(genb_sb[:], genb_d.rearrange("l c -> c l"))
            nc.sync.dma_start(outw_sb[:], outw_d)
            nc.sync.dma_start(outb_sb[:], outb_d)
            nc.sync.dma_start(bngam[:], bng_d.rearrange("l c -> (l c)")[None, :])
            nc.sync.dma_start(bnbet[:], bnb_d.rearrange("l c -> (l c)")[None, :])
            nc.sync.dma_start(
                hv[:].rearrange("p (k f) -> p k f", f=F),
                hv0_d.rearrange("(k p) f -> p k f", p=128),
            )

            blk_base = np.concatenate([[0], np.cumsum(nblk_per_chunk)])

            for l in range(L):
                # ===== Phase A: BN stats =====
                ps_sum = psA.tile([1, F], dt.float32, name="ps_sum", tag="acc")
                ps_sq = psA.tile([1, F], dt.float32, name="ps_sq", tag="acc")
                for c in range(NCHUNK):
                    hv_c = hv[:, c * F : (c + 1) * F]
                    sq = wf32.tile([128, F], dt.float32, name="sq", tag="sq")
                    nc.scalar.activation(sq[:], hv_c, AF.Square)
                    nc.tensor.matmul(
                        ps_sum[:], ones_col[:], hv_c,
                        start=(c == 0), stop=(c == NCHUNK - 1),
                    )
                    nc.tensor.matmul(
                        ps_sq[:], ones_col[:], sq[:],
                        start=(c == 0), stop=(c == NCHUNK - 1),
                    )
                s_c = small.tile([1, C], dt.float32, name="s_c", tag="st")
                q_c = small.tile([1, C], dt.float32, name="q_c", tag="st2")
                nc.vector.reduce_sum(
                    s_c[:], ps_sum[:].rearrange("one (p c) -> one c p", c=C),
                    axis=mybir.AxisListType.X,
                )
                nc.vector.reduce_sum(
                    q_c[:], ps_sq[:].rearrange("one (p c) -> one c p", c=C),
                    axis=mybir.AxisListType.X,
                )
                bn_pack = small.tile([1, 128], dt.float32, name="bn_pack", tag="bnp")
                nc.vector.tensor_copy(bn_pack[:, 0:C], s_c[:])
                nc.vector.tensor_copy(bn_pack[:, C : 2 * C], q_c[:])
                nc.sync.dma_start(bn_in[:], bn_pack[:])
                nc.gpsimd.collective_compute(
                    "AllReduce",
                    ALU.add,
                    replica_groups=[list(range(NCORES))],
                    ins=[bn_in.opt()],
                    outs=[bn_outs[l].opt()],
                )
                bn_g = small.tile([1, 128], dt.float32, name="bn_g", tag="bng")
                nc.sync.dma_start(bn_g[:], bn_outs[l][:])
                mu = small.tile([1, C], dt.float32, name="mu", tag="mu")
                ex2 = small.tile([1, C], dt.float32, name="ex2", tag="ex2")
                nc.vector.tensor_scalar_mul(mu[:], bn_g[:, 0:C], 1.0 / BN_COUNT)
                nc.vector.tensor_scalar_mul(ex2[:], bn_g[:, C : 2 * C], 1.0 / BN_COUNT)
                var = small.tile([1, C], dt.float32, name="var", tag="var")
                nc.vector.tensor_mul(var[:], mu[:], mu[:])
                nc.vector.tensor_sub(var[:], ex2[:], var[:])
                sd = small.tile([1, C], dt.float32, name="sd", tag="sd")
                nc.scalar.activation(sd[:], var[:], AF.Sqrt, bias=eps_bn[:])
                rstd = small.tile([1, C], dt.float32, name="rstd", tag="rstd")
                nc.vector.reciprocal(rstd[:], sd[:])
                a_c = small.tile([1, C], dt.float32, name="a_c", tag="ac")
                b_c = small.tile([1, C], dt.float32, name="b_c", tag="bc")
                nc.vector.tensor_mul(a_c[:], bngam[:, l * C : (l + 1) * C], rstd[:])
                nc.vector.tensor_mul(b_c[:], mu[:], a_c[:])
                nc.vector.tensor_sub(b_c[:], bnbet[:, l * C : (l + 1) * C], b_c[:])
                a_row = small.tile([1, F], dt.float32, name="a_row", tag="arow")
                b_row = small.tile([1, F], dt.float32, name="b_row", tag="brow")
                for p in range(PS):
                    nc.vector.tensor_copy(a_row[:, p * C : (p + 1) * C], a_c[:])
                    nc.vector.tensor_copy(b_row[:, p * C : (p + 1) * C], b_c[:])
                ps_ab = psB.tile([128, F], dt.float32, name="ps_ab", tag="lin")
                nc.tensor.matmul(ps_ab[:], ones_row[:], a_row[:], start=True, stop=True)
                nc.scalar.activation(a_bc[:], ps_ab[:], AF.Identity)
                ps_ab2 = psB.tile([128, F], dt.float32, name="ps_ab2", tag="lin")
                nc.tensor.matmul(
                    ps_ab2[:], ones_row[:], b_row[:], start=True, stop=True
                )
                nc.scalar.activation(b_bc[:], ps_ab2[:], AF.Identity)

                # ===== Phase B: h1 = relu(a*hv + b) =====
                for c in range(NCHUNK):
                    hv_c = hv[:, c * F : (c + 1) * F]
                    h1_c = h1f[:, c * F : (c + 1) * F]
                    z = wf32.tile([128, F], dt.float32, name="z", tag="z")
                    nc.vector.tensor_mul(z[:], hv_c, a_bc[:])
                    nc.vector.tensor_add(z[:], z[:], b_bc[:])
                    nc.vector.tensor_scalar_max(h1_c, z[:], 0.0)
                    h1h = edgep.tile([128, F], dt.float16, name="h1h", tag="h1h")
                    nc.vector.tensor_copy(h1h[:], h1_c)
                    nc.sync.dma_start(h1t[c * 128 : (c + 1) * 128, :], h1h[:])
                    if DEBUG and l == 0:
                        nc.sync.dma_start(dbg_h1[c * 128 : (c + 1) * 128, :], h1h[:])

                # ===== Phase C: edge aggregation + linear + residual =====
                loaded_group = -1
                for c in range(NCHUNK):
                    ps_s = psA.tile([128, F], dt.float32, name="ps_s", tag="acc")
                    ps_n = psA.tile([128, F], dt.float32, name="ps_n", tag="acc")
                    g0, g1 = int(blk_base[c]), int(blk_base[c + 1])
                    for g in range(g0, g1):
                        t, j = divmod(g, G)
                        if t != loaded_group:
                            loaded_group = t
                            he_t = edgep.tile(
                                [128, G * F], dt.float16, name="he_t", tag="he"
                            )
                            hs_t = edgep.tile(
                                [128, G * F], dt.float16, name="hs_t", tag="hs"
                            )
                            m_t = edgep.tile(
                                [128, G * F], dt.float16, name="m_t", tag="m"
                            )
                            e_t = edgep.tile(
                                [128, G * F], dt.float16, name="e_t", tag="e"
                            )
                            n_t = edgep.tile(
                                [128, G * F], dt.float16, name="n_t", tag="n"
                            )
                            nc.sync.dma_start(he_t[:], he_d[t])
                            for jj in range(G):
                                nc.gpsimd.indirect_dma_start(
                                    out=hs_t[:, jj * F : (jj + 1) * F],
                                    out_offset=None,
                                    in_=h1t[:, :],
                                    in_offset=bass.IndirectOffsetOnAxis(
                                        ap=gidx_sb[:, t * G + jj : t * G + jj + 1],
                                        axis=0,
                                    ),
                                )
                            if DEBUG and l == 0 and t == 0:
                                nc.sync.dma_start(dbg_gath0[:], hs_t[:])
                            nc.vector.tensor_add(m_t[:], hs_t[:], he_t[:])
                            nc.vector.tensor_scalar_max(m_t[:], m_t[:], 0.0)
                            nc.scalar.activation(e_t[:], m_t[:], AF.Exp, bias=ebias[:])
                            nc.vector.tensor_mul(n_t[:], m_t[:], e_t[:])
                        S = edgep.tile([128, 128], dt.float16, name="S", tag="S")
                        nc.vector.tensor_tensor(
                            out=S[:],
                            in0=ldst_sb[:, g : g + 1].to_broadcast([128, 128]),
                            in1=iota_f[:],
                            op=ALU.is_equal,
                        )
                        sl = slice(j * F, (j + 1) * F)
                        nc.tensor.matmul(
                            ps_s[:], S[:], e_t[:, sl],
                            start=(g == g0), stop=(g == g1 - 1),
                        )
                        nc.tensor.matmul(
                            ps_n[:], S[:], n_t[:, sl],
                            start=(g == g0), stop=(g == g1 - 1),
                        )
                    hv_c = hv[:, c * F : (c + 1) * F]
                    h1_c = h1f[:, c * F : (c + 1) * F]
                    sadj = wf32.tile([128, F], dt.float32, name="sadj", tag="sadj")
                    nc.scalar.activation(sadj[:], ps_s[:], AF.Identity, bias=s_floor[:])
                    rec = wf32.tile([128, F], dt.float32, name="rec", tag="rec")
                    nc.vector.reciprocal(rec[:], sadj[:])
                    feats = wf32.tile([128, F], dt.float32, name="feats", tag="feats")
                    nc.vector.tensor_mul(feats[:], ps_n[:], rec[:])
                    nc.vector.scalar_tensor_tensor(
                        feats[:], feats[:], GEN_EPS, h1_c,
                        op0=ALU.add, op1=ALU.add,
                    )
                    if DEBUG and l == 0 and c == 0:
                        nc.sync.dma_start(dbg_feats0[:], feats[:])
                    for p in range(PS):
                        slp = slice(p * C, (p + 1) * C)
                        ps_t = psB.tile([C, 128], dt.float32, name="ps_t", tag="lin")
                        nc.tensor.transpose(ps_t[:], feats[:, slp], ident[:])
                        fT = wf32.tile([C, 128], dt.float32, name="fT", tag="fT")
                        nc.scalar.activation(fT[:], ps_t[:], AF.Identity)
                        ps_o = psB.tile([C, 128], dt.float32, name="ps_o", tag="lin")
                        nc.tensor.matmul(
                            ps_o[:], genw_sb[:, l * C : (l + 1) * C], fT[:],
                            start=True, stop=True,
                        )
                        oT = wf32.tile([C, 128], dt.float32, name="oT", tag="oT")
                        nc.scalar.activation(
                            oT[:], ps_o[:], AF.Identity, bias=genb_sb[:, l : l + 1]
                        )
                        ps_r = psB.tile([128, C], dt.float32, name="ps_r", tag="lin")
                        nc.tensor.transpose(ps_r[:], oT[:], ident[:C, :C])
                        nc.vector.tensor_add(hv_c[:, slp], hv_c[:, slp], ps_r[:])
                    if c == NCHUNK - 1:
                        nc.vector.tensor_scalar_mul(hv_c, hv_c, pad_mask[:])
                if DEBUG and l == 0:
                    nc.sync.dma_start(dbg_hv1, hv[:])

            # ===== Final: mean over W (group AllReduce), h_g, output linear =====
            for c in range(NCHUNK):
                mwh = edgep.tile([128, F], dt.float16, name="mwh", tag="mwh")
                nc.vector.tensor_scalar_mul(mwh[:], hv[:, c * F : (c + 1) * F], 0.25)
                nc.sync.dma_start(mw_in[c * 128 : (c + 1) * 128, :], mwh[:])
            nc.gpsimd.collective_compute(
                "AllReduce",
                mybir.AluOpType.add,
                replica_groups=[[0, 1, 2, 3], [4, 5, 6, 7]],
                ins=[mw_in.opt()],
                outs=[mw_out.opt()],
            )
            for c in range(NCHUNK):
                hv_c = hv[:, c * F : (c + 1) * F]
                mean_h = edgep.tile([128, F], dt.float16, name="mean_h", tag="meanh")
                nc.sync.dma_start(mean_h[:], mw_out[c * 128 : (c + 1) * 128, :])
                mean_f = wf32.tile([128, F], dt.float32, name="mean_f", tag="meanf")
                nc.vector.tensor_copy(mean_f[:], mean_h[:])
                hg = wf32.tile([128, F], dt.float32, name="hg", tag="hg")
                nc.vector.tensor_mul(hg[:], hv_c, mean_f[:])
                outsb = wf32.tile([128, O * PS], dt.float32, name="outsb", tag="outsb")
                for p in range(PS):
                    ps_t2 = psB.tile([C, 128], dt.float32, name="ps_t2", tag="lin")
                    nc.tensor.transpose(ps_t2[:], hg[:, p * C : (p + 1) * C], ident[:])
                    fT2 = wf32.tile([C, 128], dt.float32, name="fT2", tag="fT")
                    nc.scalar.activation(fT2[:], ps_t2[:], AF.Identity)
                    ps_o2 = psB.tile([O, 128], dt.float32, name="ps_o2", tag="lin")
                    nc.tensor.matmul(
                        ps_o2[:], outw_sb[:], fT2[:], start=True, stop=True
                    )
                    oT2 = wf32.tile([O, 128], dt.float32, name="oT2", tag="oT2")
                    nc.scalar.activation(
                        oT2[:], ps_o2[:], AF.Identity, bias=outb_sb[:]
                    )
                    ps_r2 = psB.tile([128, O], dt.float32, name="ps_r2", tag="lin")
                    nc.tensor.transpose(ps_r2[:], oT2[:], ident[:O, :O])
                    nc.vector.tensor_copy(outsb[:, p * O : (p + 1) * O], ps_r2[:])
                nc.sync.dma_start(out_d[c * 128 : (c + 1) * 128, :], outsb[:])

    nc.compile()
    return nc


def kernel(
    node_feats, edge_feats, src, dst, bn_gamma, bn_beta, gen_W, gen_b, out_W, out_b
):
    from concourse import bass_utils

    in_maps, nblk_per_chunk, pos_sets = _host_prep(node_feats, edge_feats, src, dst)

    key = tuple(nblk_per_chunk)
    if key not in _cache:
        _cache[key] = _build_program(nblk_per_chunk)
    nc = _cache[key]

    bnga = np.ascontiguousarray(np.asarray(bn_gamma, np.float32))
    bnbe = np.ascontiguousarray(np.asarray(bn_beta, np.float32))
    genw = np.ascontiguousarray(np.asarray(gen_W, np.float32).reshape(L * C, C))
    genb = np.ascontiguousarray(np.asarray(gen_b, np.float32))
    outw = np.ascontiguousarray(np.asarray(out_W, np.float32))
    outb = np.ascontiguousarray(np.asarray(out_b, np.float32).reshape(O, 1))
    for m in in_maps:
        m["bn_gamma"] = bnga
        m["bn_beta"] = bnbe
        m["gen_W"] = genw
        m["gen_b"] = genb
        m["out_W"] = outw
        m["out_b"] = outb

    res = bass_utils.run_bass_kernel_spmd(nc, in_maps, core_ids=list(range(NCORES)))

    out = np.zeros((N_NODES, O, H, W), np.float32)
    for k in range(NCORES):
        o_k = np.asarray(res.results[k]["out_sh"])[:N_NODES].reshape(N_NODES, PS, O)
        for j, pos in enumerate(pos_sets[k]):
            out[:, :, pos // 4, pos % 4] = o_k[:, j, :]
    return out


# revision 12
# speedup vs baseline: 2.4185x; 2.4185x over previous
"""DeepGCN (GENConv softmax-aggr) Trainium2 kernel, 8-core SPMD.

Sharding: by spatial feature position (H*W = 48 -> 6 per core).
Core k owns positions {h*4 + (k%4) : h in [6*(k//4), 6*(k//4)+6)}.
All nodes are local on every core; per-layer BN stats use a tiny (512B)
AllReduce, and the final mean-over-W pooling uses one fp16 AllReduce
within each h-half group of 4 cores.
"""

import sys
import numpy as np

for p in ("/opt/trn_rl_repo",):
    if p not in sys.path:
        sys.path.insert(0, p)

# ---- problem constants (hardcoded per spec) ----
N_NODES = 5000
N_EDGES = 50000
C = 64
H, W = 12, 4
HW = H * W          # 48
PS = 6              # positions per core
F = C * PS          # 384 features per core-slice  (layout f = p*64 + c)
O = 12
L = 3
NCORES = 8
NCHUNK = 40         # dst-node chunks of 128
NODES_PAD = NCHUNK * 128   # 5120
G = 4               # edge blocks per DMA/compute group (128 edges each)
BN_EPS = 1e-5
GEN_EPS = 1e-7
EXP_BIAS = -4.0     # e' = exp(m + EXP_BIAS); cancels in softmax, keeps fp16 safe
BN_COUNT = float(N_NODES * HW)

_cache = {}
DEBUG = False
NL = L          # layers to build (bench knob)
FINAL = True    # include final phase (bench knob)


def _host_prep(node_feats, edge_feats, src, dst):
    """Sort/pad edges by dst chunk, build per-core input maps."""
    src = np.asarray(src).astype(np.int64)
    dst = np.asarray(dst).astype(np.int64)
    nf = np.asarray(node_feats, dtype=np.float32).reshape(N_NODES, C, HW)
    ef = np.asarray(edge_feats, dtype=np.float32).reshape(N_EDGES, C, HW)

    order = np.argsort(dst, kind="stable")
    chunk_of = dst[order] // 128

    blk_edges = []          # original-edge indices, -1 for pads
    nblk_per_chunk = []
    for c in range(NCHUNK):
        sel = order[chunk_of == c]
        nb = max(1, (len(sel) + 127) // 128)
        pad = nb * 128 - len(sel)
        blk_edges.append(np.concatenate([sel, np.full(pad, -1, np.int64)]))
        nblk_per_chunk.append(nb)
    nblk_tot = sum(nblk_per_chunk)
    extra = (-nblk_tot) % G
    if extra:
        blk_edges[-1] = np.concatenate(
            [blk_edges[-1], np.full(extra * 128, -1, np.int64)]
        )
        nblk_per_chunk[-1] += extra
        nblk_tot += extra

    eidx = np.concatenate(blk_edges)          # [nblk_tot*128]
    valid = eidx >= 0
    e_src = np.where(valid, src[np.maximum(eidx, 0)], 0).astype(np.int32)
    chunk_id = np.concatenate(
        [np.full(nblk_per_chunk[c] * 128, c, np.int64) for c in range(NCHUNK)]
    )
    e_ldst = np.where(
        valid, dst[np.maximum(eidx, 0)] - chunk_id * 128, -1
    ).astype(np.float32)

    NBLK = nblk_tot
    NG = NBLK // G
    gidx_d = np.ascontiguousarray(e_src.reshape(NBLK, 128).T)    # [128, NBLK] int32
    ldst_d = np.ascontiguousarray(e_ldst.reshape(NBLK, 128).T)   # [128, NBLK] f32

    pos_sets = []
    for k in range(NCORES):
        w = k % 4
        h0 = 6 * (k // 4)
        pos_sets.append(np.array([h * 4 + w for h in range(h0, h0 + 6)]))

    ev = eidx[valid]
    in_maps = []
    for k in range(NCORES):
        P_k = pos_sets[k]
        hv0 = np.zeros((NODES_PAD, F), np.float32)
        hv0[:N_NODES] = nf[:, :, P_k].transpose(0, 2, 1).reshape(N_NODES, F)
        he_s = np.zeros((NBLK * 128, F), np.float16)
        he_s[valid] = (
            ef[ev][:, :, P_k].transpose(0, 2, 1).reshape(len(ev), F).astype(np.float16)
        )
        he_pm = (
            he_s.reshape(NG, G, 128, F).transpose(0, 2, 1, 3).reshape(NG, 128, G * F)
        )
        in_maps.append(
            {
                "hv0": hv0,
                "he_pm": np.ascontiguousarray(he_pm),
                "gidx": gidx_d,
                "ldst": ldst_d,
            }
        )
    return in_maps, nblk_per_chunk, pos_sets


def _build_program(nblk_per_chunk):
    import concourse.bacc as bacc
    import concourse.tile as tile
    from concourse import bass, mybir
    from concourse.masks import make_identity

    dt = mybir.dt
    AF = mybir.ActivationFunctionType
    ALU = mybir.AluOpType
    NBLK = sum(nblk_per_chunk)
    NG = NBLK // G

    nc = bacc.Bacc(
        "TRN2",
        target_bir_lowering=False,
        debug=False,
        enable_asserts=False,
        num_devices=NCORES,
    )

    hv0_d = nc.dram_tensor("hv0", [NODES_PAD, F], dt.float32, kind="ExternalInput").ap()
    he_d = nc.dram_tensor(
        "he_pm", [NG, 128, G * F], dt.float16, kind="ExternalInput"
    ).ap()
    gidx_d = nc.dram_tensor("gidx", [128, NBLK], dt.int32, kind="ExternalInput").ap()
    ldst_d = nc.dram_tensor("ldst", [128, NBLK], dt.float32, kind="ExternalInput").ap()
    bng_d = nc.dram_tensor("bn_gamma", [L, C], dt.float32, kind="ExternalInput").ap()
    bnb_d = nc.dram_tensor("bn_beta", [L, C], dt.float32, kind="ExternalInput").ap()
    genw_d = nc.dram_tensor("gen_W", [L * C, C], dt.float32, kind="ExternalInput").ap()
    genb_d = nc.dram_tensor("gen_b", [L, C], dt.float32, kind="ExternalInput").ap()
    outw_d = nc.dram_tensor("out_W", [C, O], dt.float32, kind="ExternalInput").ap()
    outb_d = nc.dram_tensor("out_b", [O, 1], dt.float32, kind="ExternalInput").ap()
    out_d = nc.dram_tensor(
        "out_sh", [NODES_PAD, O * PS], dt.float32, kind="ExternalOutput"
    ).ap()
    if DEBUG:
        dbg_h1 = nc.dram_tensor(
            "dbg_h1", [NODES_PAD, F], dt.float16, kind="ExternalOutput"
        ).ap()
        dbg_hv1 = nc.dram_tensor(
            "dbg_hv1", [128, NCHUNK * F], dt.float32, kind="ExternalOutput"
        ).ap()
        dbg_feats0 = nc.dram_tensor(
            "dbg_feats0", [128, F], dt.float32, kind="ExternalOutput"
        ).ap()
        dbg_gath0 = nc.dram_tensor(
            "dbg_gath0", [128, G * F], dt.float16, kind="ExternalOutput"
        ).ap()

    with tile.TileContext(nc) as tc:
        with (
            tc.tile_pool(name="dram", bufs=1, space="DRAM") as dramp,
            tc.tile_pool(name="resident", bufs=1) as res,
            tc.tile_pool(name="wf32", bufs=2) as wf32,
            tc.tile_pool(name="edge", bufs=2) as edgep,
            tc.tile_pool(name="small", bufs=2) as small,
            tc.tile_pool(name="psA", bufs=4, space="PSUM") as psA,
            tc.tile_pool(name="psB", bufs=3, space="PSUM") as psB,
        ):
            h1t = dramp.tile([NODES_PAD, F], dt.float16, name="h1t")
            bn_in = dramp.tile([1, 128], dt.float32, name="bn_in")
            bn_outs = [
                dramp.tile(
                    [1, 128], dt.float32, addr_space="Shared",
                    name=f"bn_out{l}", tag=f"bn_out{l}",
                )
                for l in range(L)
            ]
            mw_in = dramp.tile([NODES_PAD, F], dt.float16, name="mw_in")
            mw_out = dramp.tile([NODES_PAD, F], dt.float16, name="mw_out")

            hv = res.tile([128, NCHUNK * F], dt.float32, name="hv")
            h1f = res.tile([128, NCHUNK * F], dt.float32, name="h1f")
            gidx_sb = res.tile([128, NBLK], dt.int32, name="gidx_sb")
            ldst_sb = res.tile([128, NBLK], dt.float32, name="ldst_sb")
            ident = res.tile([128, 128], dt.float32, name="ident")
            iota_f = res.tile([128, 128], dt.float32, name="iota_f")
            ones_col = res.tile([128, 1], dt.float32, name="ones_col")
            ones_row = res.tile([1, 128], dt.float32, name="ones_row")
            genw_sb = res.tile([C, L * C], dt.float32, name="genw_sb")
            genb_sb = res.tile([C, L], dt.float32, name="genb_sb")
            outw_sb = res.tile([C, O], dt.float32, name="outw_sb")
            outb_sb = res.tile([O, 1], dt.float32, name="outb_sb")
            bngam = res.tile([1, L * C], dt.float32, name="bngam")
            bnbet = res.tile([1, L * C], dt.float32, name="bnbet")
            a_bc = res.tile([128, F], dt.float32, name="a_bc")
            b_bc = res.tile([128, F], dt.float32, name="b_bc")
            eps_bn = res.tile([1, 1], dt.float32, name="eps_bn")
            pad_mask = res.tile([128, 1], dt.float32, name="pad_mask")
            ebias = res.tile([128, 1], dt.float32, name="ebias")
            s_floor = res.tile([128, 1], dt.float32, name="s_floor")

            make_identity(nc, ident[:])
            iota_i = small.tile([128, 128], dt.int32, name="iota_i", tag="ioi")
            nc.gpsimd.iota(iota_i[:], pattern=[[1, 128]], base=0, channel_multiplier=0)
            nc.vector.tensor_copy(iota_f[:], iota_i[:])
            nc.gpsimd.memset(ones_col[:], 1.0)
            nc.gpsimd.memset(ones_row[:], 1.0)
            nc.gpsimd.memset(eps_bn[:], BN_EPS)
            nc.gpsimd.memset(ebias[:], EXP_BIAS)
            nc.gpsimd.memset(s_floor[:], 1e-30)
            iota_c = small.tile([128, 1], dt.int32, name="iota_c", tag="ioc")
            nc.gpsimd.iota(iota_c[:], pattern=[[1, 1]], base=0, channel_multiplier=1)
            nc.vector.tensor_scalar(
                out=pad_mask[:], in0=iota_c[:], scalar1=8, scalar2=None,
                op0=ALU.is_lt,
            )

            nc.sync.dma_start(gidx_sb[:], gidx_d)
            nc.sync.dma_start(ldst_sb[:], ldst_d)
            for l in range(L):
                nc.sync.dma_start(
                    genw_sb[:, l * C : (l + 1) * C], genw_d[l * C : (l + 1) * C, :]
                )
            nc.sync.dma_start(genb_sb[:], genb_d.rearrange("l c -> c l"))
            nc.sync.dma_start(outw_sb[:], outw_d)
            nc.sync.dma_start(outb_sb[:], outb_d)
            nc.sync.dma_start(bngam[:], bng_d.rearrange("l c -> (l c)")[None, :])
            nc.sync.dma_start(bnbet[:], bnb_d.rearrange("l c -> (l c)")[None, :])
            nc.sync.dma_start(
                hv[:].rearrange("p (k f) -> p k f", f=F),
                hv0_d.rearrange("(k p) f -> p k f", p=128),
            )

            blk_base = np.concatenate([[0], np.cumsum(nblk_per_chunk)])

            for l in range(NL):
                # ===== Phase A: BN stats =====
                ps_sum = psA.tile([1, F], dt.float32, name="ps_sum", tag="acc")
                ps_sq = psA.tile([1, F], dt.float32, name="ps_sq", tag="acc")
                for c in range(NCHUNK):
                    hv_c = hv[:, c * F : (c + 1) * F]
                    sq = wf32.tile([128, F], dt.float32, name="sq", tag="sq")
                    nc.scalar.activation(sq[:], hv_c, AF.Square)
                    nc.tensor.matmul(
                        ps_sum[:], ones_col[:], hv_c,
                        start=(c == 0), stop=(c == NCHUNK - 1),
                    )
                    nc.tensor.matmul(
                        ps_sq[:], ones_col[:], sq[:],
                        start=(c == 0), stop=(c == NCHUNK - 1),
                    )
                s_c = small.tile([1, C], dt.float32, name="s_c", tag="st")
                q_c = small.tile([1, C], dt.float32, name="q_c", tag="st2")
                nc.vector.reduce_sum(
                    s_c[:], ps_sum[:].rearrange("one (p c) -> one c p", c=C),
                    axis=mybir.AxisListType.X,
                )
                nc.vector.reduce_sum(
                    q_c[:], ps_sq[:].rearrange("one (p c) -> one c p", c=C),
                    axis=mybir.AxisListType.X,
                )
                bn_pack = small.tile([1, 128], dt.float32, name="bn_pack", tag="bnp")
                nc.vector.tensor_copy(bn_pack[:, 0:C], s_c[:])
                nc.vector.tensor_copy(bn_pack[:, C : 2 * C], q_c[:])
                nc.sync.dma_start(bn_in[:], bn_pack[:])
                nc.gpsimd.collective_compute(
                    "AllReduce",
                    ALU.add,
                    replica_groups=[list(range(NCORES))],
                    ins=[bn_in.opt()],
                    outs=[bn_outs[l].opt()],
                )
                bn_g = small.tile([1, 128], dt.float32, name="bn_g", tag="bng")
                nc.sync.dma_start(bn_g[:], bn_outs[l][:])
                mu = small.tile([1, C], dt.float32, name="mu", tag="mu")
                ex2 = small.tile([1, C], dt.float32, name="ex2", tag="ex2")
                nc.vector.tensor_scalar_mul(mu[:], bn_g[:, 0:C], 1.0 / BN_COUNT)
                nc.vector.tensor_scalar_mul(ex2[:], bn_g[:, C : 2 * C], 1.0 / BN_COUNT)
                var = small.tile([1, C], dt.float32, name="var", tag="var")
                nc.vector.tensor_mul(var[:], mu[:], mu[:])
                nc.vector.tensor_sub(var[:], ex2[:], var[:])
                sd = small.tile([1, C], dt.float32, name="sd", tag="sd")
                nc.scalar.activation(sd[:], var[:], AF.Sqrt, bias=eps_bn[:])
                rstd = small.tile([1, C], dt.float32, name="rstd", tag="rstd")
                nc.vector.reciprocal(rstd[:], sd[:])
                a_c = small.tile([1, C], dt.float32, name="a_c", tag="ac")
                b_c = small.tile([1, C], dt.float32, name="b_c", tag="bc")
                nc.vector.tensor_mul(a_c[:], bngam[:, l * C : (l + 1) * C], rstd[:])
                nc.vector.tensor_mul(b_c[:], mu[:], a_c[:])
                nc.vector.tensor_sub(b_c[:], bnbet[:, l * C : (l + 1) * C], b_c[:])
                a_row = small.tile([1, F], dt.float32, name="a_row", tag="arow")
                b_row = small.tile([1, F], dt.float32, name="b_row", tag="brow")
                for p in range(PS):
                    nc.vector.tensor_copy(a_row[:, p * C : (p + 1) * C], a_c[:])
                    nc.vector.tensor_copy(b_row[:, p * C : (p + 1) * C], b_c[:])
                ps_ab = psB.tile([128, F], dt.float32, name="ps_ab", tag="lin")
                nc.tensor.matmul(ps_ab[:], ones_row[:], a_row[:], start=True, stop=True)
                nc.scalar.activation(a_bc[:], ps_ab[:], AF.Identity)
                ps_ab2 = psB.tile([128, F], dt.float32, name="ps_ab2", tag="lin")
                nc.tensor.matmul(
                    ps_ab2[:], ones_row[:], b_row[:], start=True, stop=True
                )
                nc.scalar.activation(b_bc[:], ps_ab2[:], AF.Identity)

                # ===== Phase B: h1 = relu(a*hv + b) =====
                for c in range(NCHUNK):
                    hv_c = hv[:, c * F : (c + 1) * F]
                    h1_c = h1f[:, c * F : (c + 1) * F]
                    z = wf32.tile([128, F], dt.float32, name="z", tag="z")
                    nc.vector.tensor_mul(z[:], hv_c, a_bc[:])
                    nc.vector.tensor_add(z[:], z[:], b_bc[:])
                    nc.vector.tensor_scalar_max(h1_c, z[:], 0.0)
                    h1h = edgep.tile([128, F], dt.float16, name="h1h", tag="h1h")
                    nc.vector.tensor_copy(h1h[:], h1_c)
                    nc.sync.dma_start(h1t[c * 128 : (c + 1) * 128, :], h1h[:])
                    if DEBUG and l == 0:
                        nc.sync.dma_start(dbg_h1[c * 128 : (c + 1) * 128, :], h1h[:])

                # ===== Phase C: edge aggregation + linear + residual =====
                loaded_group = -1
                for c in range(NCHUNK):
                    ps_s = psA.tile([128, F], dt.float32, name="ps_s", tag="acc")
                    ps_n = psA.tile([128, F], dt.float32, name="ps_n", tag="acc")
                    g0, g1 = int(blk_base[c]), int(blk_base[c + 1])
                    for g in range(g0, g1):
                        t, j = divmod(g, G)
                        if t != loaded_group:
                            loaded_group = t
                            he_t = edgep.tile(
                                [128, G * F], dt.float16, name="he_t", tag="he"
                            )
                            hs_t = edgep.tile(
                                [128, G * F], dt.float16, name="hs_t", tag="hs"
                            )
                            m_t = edgep.tile(
                                [128, G * F], dt.float16, name="m_t", tag="m"
                            )
                            e_t = edgep.tile(
                                [128, G * F], dt.float16, name="e_t", tag="e"
                            )
                            n_t = edgep.tile(
                                [128, G * F], dt.float16, name="n_t", tag="n"
                            )
                            nc.sync.dma_start(he_t[:], he_d[t])
                            for jj in range(G):
                                nc.gpsimd.indirect_dma_start(
                                    out=hs_t[:, jj * F : (jj + 1) * F],
                                    out_offset=None,
                                    in_=h1t[:, :],
                                    in_offset=bass.IndirectOffsetOnAxis(
                                        ap=gidx_sb[:, t * G + jj : t * G + jj + 1],
                                        axis=0,
                                    ),
                                )
                            if DEBUG and l == 0 and t == 0:
                                nc.sync.dma_start(dbg_gath0[:], hs_t[:])
                            nc.vector.tensor_add(m_t[:], hs_t[:], he_t[:])
                            nc.vector.tensor_scalar_max(m_t[:], m_t[:], 0.0)
                            nc.scalar.activation(e_t[:], m_t[:], AF.Exp, bias=ebias[:])
                            nc.vector.tensor_mul(n_t[:], m_t[:], e_t[:])
                        S = edgep.tile([128, 128], dt.float16, name="S", tag="S")
                        nc.vector.tensor_tensor(
                            out=S[:],
                            in0=ldst_sb[:, g : g + 1].to_broadcast([128, 128]),
                            in1=iota_f[:],
                            op=ALU.is_equal,
                        )
                        sl = slice(j * F, (j + 1) * F)
                        nc.tensor.matmul(
                            ps_s[:], S[:], e_t[:, sl],
                            start=(g == g0), stop=(g == g1 - 1),
                        )
                        nc.tensor.matmul(
                            ps_n[:], S[:], n_t[:, sl],
                            start=(g == g0), stop=(g == g1 - 1),
                        )
                    hv_c = hv[:, c * F : (c + 1) * F]
                    h1_c = h1f[:, c * F : (c + 1) * F]
                    sadj = wf32.tile([128, F], dt.float32, name="sadj", tag="sadj")
                    nc.scalar.activation(sadj[:], ps_s[:], AF.Identity, bias=s_floor[:])
                    rec = wf32.tile([128, F], dt.float32, name="rec", tag="rec")
                    nc.vector.reciprocal(rec[:], sadj[:])
                    feats = wf32.tile([128, F], dt.float32, name="feats", tag="feats")
                    nc.vector.tensor_mul(feats[:], ps_n[:], rec[:])
                    nc.vector.scalar_tensor_tensor(
                        feats[:], feats[:], GEN_EPS, h1_c,
                        op0=ALU.add, op1=ALU.add,
                    )
                    if DEBUG and l == 0 and c == 0:
                        nc.sync.dma_start(dbg_feats0[:], feats[:])
                    for p in range(PS):
                        slp = slice(p * C, (p + 1) * C)
                        ps_t = psB.tile([C, 128], dt.float32, name="ps_t", tag="lin")
                        nc.tensor.transpose(ps_t[:], feats[:, slp], ident[:])
                        fT = wf32.tile([C, 128], dt.float32, name="fT", tag="fT")
                        nc.scalar.activation(fT[:], ps_t[:], AF.Identity)
                        ps_o = psB.tile([C, 128], dt.float32, name="ps_o", tag="lin")
                        nc.tensor.matmul(
                            ps_o[:], genw_sb[:, l * C : (l + 1) * C], fT[:],
                            start=True, stop=True,
                        )
                        oT = wf32.tile([C, 128], dt.float32, name="oT", tag="oT")
                        nc.scalar.activation(
                            oT[:], ps_o[:], AF.Identity, bias=genb_sb[:, l : l + 1]
                        )
                        ps_r = psB.tile([128, C], dt.float32, name="ps_r", tag="lin")
                        nc.tensor.transpose(ps_r[:], oT[:], ident[:C, :C])
                        nc.vector.tensor_add(hv_c[:, slp], hv_c[:, slp], ps_r[:])
                    if c == NCHUNK - 1:
                        nc.vector.tensor_scalar_mul(hv_c, hv_c, pad_mask[:])
                if DEBUG and l == 0:
                    nc.sync.dma_start(dbg_hv1, hv[:])

            # ===== Final: mean over W (group AllReduce), h_g, output linear =====
            for c in range(NCHUNK if FINAL else 1):
                mwh = edgep.tile([128, F], dt.float16, name="mwh", tag="mwh")
                nc.vector.tensor_scalar_mul(mwh[:], hv[:, c * F : (c + 1) * F], 0.25)
                nc.sync.dma_start(mw_in[c * 128 : (c + 1) * 128, :], mwh[:])
            nc.gpsimd.collective_compute(
                "AllReduce",
                mybir.AluOpType.add,
                replica_groups=[[0, 1, 2, 3], [4, 5, 6, 7]],
                ins=[mw_in.opt()],
                outs=[mw_out.opt()],
            )
            for c in range(NCHUNK if FINAL else 1):
                hv_c = hv[:, c * F : (c + 1) * F]
                mean_h = edgep.tile([128, F], dt.float16, name="mean_h", tag="meanh")
                nc.sync.dma_start(mean_h[:], mw_out[c * 128 : (c + 1) * 128, :])
                mean_f = wf32.tile([128, F], dt.float32, name="mean_f", tag="meanf")
                nc.vector.tensor_copy(mean_f[:], mean_h[:])
                hg = wf32.tile([128, F], dt.float32, name="hg", tag="hg")
                nc.vector.tensor_mul(hg[:], hv_c, mean_f[:])
                outsb = wf32.tile([128, O * PS], dt.float32, name="outsb", tag="outsb")
                for p in range(PS):
                    ps_t2 = psB.tile([C, 128], dt.float32, name="ps_t2", tag="lin")
                    nc.tensor.transpose(ps_t2[:], hg[:, p * C : (p + 1) * C], ident[:])
                    fT2 = wf32.tile([C, 128], dt.float32, name="fT2", tag="fT")
                    nc.scalar.activation(fT2[:], ps_t2[:], AF.Identity)
                    ps_o2 = psB.tile([O, 128], dt.float32, name="ps_o2", tag="lin")
                    nc.tensor.matmul(
                        ps_o2[:], outw_sb[:], fT2[:], start=True, stop=True
                    )
                    oT2 = wf32.tile([O, 128], dt.float32, name="oT2", tag="oT2")
                    nc.scalar.activation(
                        oT2[:], ps_o2[:], AF.Identity, bias=outb_sb[:]
                    )
                    ps_r2 = psB.tile([128, O], dt.float32, name="ps_r2", tag="lin")
                    nc.tensor.transpose(ps_r2[:], oT2[:], ident[:O, :O])
                    nc.vector.tensor_copy(outsb[:, p * O : (p + 1) * O], ps_r2[:])
                nc.sync.dma_start(out_d[c * 128 : (c + 1) * 128, :], outsb[:])

    nc.compile()
    return nc


def kernel(
    node_feats, edge_feats, src, dst, bn_gamma, bn_beta, gen_W, gen_b, out_W, out_b
):
    from concourse import bass_utils

    in_maps, nblk_per_chunk, pos_sets = _host_prep(node_feats, edge_feats, src, dst)

    key = tuple(nblk_per_chunk)
    if key not in _cache:
        _cache[key] = _build_program(nblk_per_chunk)
    nc = _cache[key]

    bnga = np.ascontiguousarray(np.asarray(bn_gamma, np.float32))
    bnbe = np.ascontiguousarray(np.asarray(bn_beta, np.float32))
    genw = np.ascontiguousarray(np.asarray(gen_W, np.float32).reshape(L * C, C))
    genb = np.ascontiguousarray(np.asarray(gen_b, np.float32))
    outw = np.ascontiguousarray(np.asarray(out_W, np.float32))
    outb = np.ascontiguousarray(np.asarray(out_b, np.float32).reshape(O, 1))
    for m in in_maps:
        m["bn_gamma"] = bnga
        m["bn_beta"] = bnbe
        m["gen_W"] = genw
        m["gen_b"] = genb
        m["out_W"] = outw
        m["out_b"] = outb

    res = bass_utils.run_bass_kernel_spmd(nc, in_maps, core_ids=list(range(NCORES)))

    out = np.zeros((N_NODES, O, H, W), np.float32)
    for k in range(NCORES):
        o_k = np.asarray(res.results[k]["out_sh"])[:N_NODES].reshape(N_NODES, PS, O)
        for j, pos in enumerate(pos_sets[k]):
            out[:, :, pos // 4, pos % 4] = o_k[:, j, :]
    return out
